# revision 11
# baseline (speedup 1.0000x reference)
"""DistanceFromAnswerLoss on 8 Trainium2 NeuronCores.

out = 0.1 * sum_{b,c} mask[b,c] * exp(input[b,c])
  mask[b,c] = |c - t_b| / sqrt(sum_c (c - t_b)^2),  mask = 0 where t_b == 0

Sharding: data-parallel over the batch dim (512 rows per core); each core
emits 128 partial sums, host adds the 8*128 scalars.

Per-core pipeline (memory-bound: ~43.5us DMA floor at ~390 GB/s):
  host    : bias_b = ln(0.1) - 0.5*ln(C*(t-mu)^2 + K)  (O(B) marshalling;
            -1e4 for t==0 rows so e' == 0 in f32)
  ScalarE : e' = exp(x + bias_b) -> bf16 (the only ScalarE pass; one
            exp_and_others table load)
  VectorE : ds = (iota2k - t_rb) + 2048*m   tensor_scalar dual-op, 4x
            dt = ds & 0x7fff (bf16 sign-clear = abs)   tensor_scalar, 4x
            p  = dt * e'                               tensor_tensor, 2x
  TensorE : row-reduce via identity-stationary matmuls accumulating
            R[b, j] += p[b, 512k + j] into one PSUM bank; final DVE
            tensor_reduce collapses R[128,512] -> rs[128,1] -> DMA out.
iota is a single gpsimd [128,2048] tile; per-2048-block column offsets are
immediates in the TS.  Tile order is cw-major.  First two x-tile DMA
triggers lead the sync queue so the HBM stream starts immediately.
"""

import os
import sys
from contextlib import ExitStack

import numpy as np

sys.path.insert(0, "/opt/trn_rl_repo")

import concourse.bass as bass
import concourse.tile as tile
from concourse import bacc, mybir
from concourse.bass_utils import run_bass_kernel_spmd

B = 4096
C = 8192
N_CORES = 8
ROWS = B // N_CORES          # 512 rows per core
RB = ROWS // 128             # 4 row blocks of 128 partitions
W = 4096                     # column tile width (2 MiB DMAs)
NW = C // W
NT = RB * NW                 # 8 big tiles per core
NSTRIP = 4                   # last tile is split into NSTRIP strips
MMW = 512                    # matmul moving width (= one PSUM bank of f32)
IW = 2048                    # iota tile width; block offsets are immediates
COEFF = 0.1

MU = (C - 1) / 2.0
_S1 = (C - 1) * C // 2
_S2 = (C - 1) * C * (2 * C - 1) // 6
K = float(_S2 - _S1 * _S1 / C)   # sum_c (c-t)^2 = C*(t-MU)^2 + K

F32 = mybir.dt.float32
BF16 = mybir.dt.bfloat16
U16 = mybir.dt.uint16
Af = mybir.ActivationFunctionType
Op = mybir.AluOpType


def _build() -> bass.Bass:
    nc = bacc.Bacc("TRN2", target_bir_lowering=False, debug=False)
    x = nc.declare_dram_parameter("x", [RB, 128, C], F32, isOutput=False)
    t2 = nc.declare_dram_parameter("t2", [RB, 128, 1], F32, isOutput=False)
    bs = nc.declare_dram_parameter("bs", [RB, 128, 1], F32, isOutput=False)
    ident = nc.declare_dram_parameter("ident", [128, 128], BF16, isOutput=False)
    out = nc.declare_dram_parameter("out", [128, 1], F32, isOutput=True)

    with tile.TileContext(nc) as tc, ExitStack() as ctx:
        const_pool = ctx.enter_context(tc.tile_pool(name="const", bufs=1))
        xpool = ctx.enter_context(tc.tile_pool(name="x", bufs=4))
        epool = ctx.enter_context(tc.tile_pool(name="e", bufs=3))
        dpool = ctx.enter_context(tc.tile_pool(name="d", bufs=2))
        ppool = ctx.enter_context(tc.tile_pool(name="p", bufs=2))
        spool = ctx.enter_context(tc.tile_pool(name="s", bufs=1))
        psum_pool = ctx.enter_context(tc.tile_pool(name="ps", bufs=1, space="PSUM"))

        # --- first two x-tile DMAs lead the sync queue (cw-major order) ---
        def tile_rc(idx):
            return idx % RB, idx // RB  # rb, cw

        xts = {}
        for idx in range(2):
            rb, cw = tile_rc(idx)
            xt = xpool.tile([128, W], F32)
            nc.sync.dma_start(out=xt[:], in_=x[rb, :, cw * W:(cw + 1) * W])
            xts[idx] = xt

        # --- tiny front matter: t cols, bias cols, identity ---------------
        tcols, bcols = [], []
        for rb in range(RB):
            tc_rb = const_pool.tile([128, 1], F32, tag=f"tc{rb}")
            nc.sync.dma_start(out=tc_rb[:], in_=t2[rb])
            tcols.append(tc_rb)
            bc_rb = const_pool.tile([128, 1], F32, tag=f"bc{rb}")
            nc.sync.dma_start(out=bc_rb[:], in_=bs[rb])
            bcols.append(bc_rb)
        idt = const_pool.tile([128, 128], BF16)
        nc.sync.dma_start(out=idt[:], in_=ident[:, :])

        absmask = spool.tile([128, 1], U16)
        nc.vector.memset(absmask[:], 0x7FFF)

        iota = const_pool.tile([128, IW], BF16)
        nc.gpsimd.iota(
            iota[:], pattern=[[1, IW]], base=0, channel_multiplier=0,
            allow_small_or_imprecise_dtypes=True,
        )

        # --- main loop ----------------------------------------------------
        R = psum_pool.tile([128, MMW], F32)
        n_mm = (NT - 1) * (W // MMW) + NSTRIP * (W // NSTRIP // MMW)
        mm_i = 0

        def do_tile(xt, rb, c0, width):
            nonlocal mm_i
            et = epool.tile([128, width], BF16, tag="e" if width == W else "es")
            nc.scalar.activation(et[:], xt[:], Af.Exp, bias=bcols[rb][:])
            ds = dpool.tile([128, width], BF16, tag="d0" if width == W else "d0s")
            # per-2048 block: ds_j = (j - t_rb) + block_base
            for o in range(0, width, IW):
                bw = min(IW, width - o)
                io = (c0 + o) % IW
                nc.vector.tensor_scalar(
                    ds[:, o:o + bw], iota[:, io:io + bw], tcols[rb][:],
                    float(c0 + o - io), op0=Op.subtract, op1=Op.add,
                )
            # |x| for bf16 = clear the sign bit on the uint16 view
            dt = dpool.tile([128, width], BF16, tag="d" if width == W else "dss")
            # CoreSim rejects non-f32 ptr scalars; HW rejects nothing here —
            # use an int immediate under ABS_IMM=1 (sim), the ptr tile on HW.
            mask_arg = 0x7FFF if os.environ.get("ABS_IMM") else absmask[:]
            nc.vector.tensor_scalar(
                dt[:].bitcast(U16), ds[:].bitcast(U16), mask_arg, None,
                op0=Op.bitwise_and,
            )
            pt = ppool.tile([128, width], BF16, tag="p" if width == W else "ps")
            nc.vector.tensor_tensor(pt[:], dt[:], et[:], op=Op.mult)
            for j in range(width // MMW):
                nc.tensor.matmul(
                    R[:], idt[:], pt[:, j * MMW:(j + 1) * MMW],
                    start=(mm_i == 0), stop=(mm_i == n_mm - 1),
                )
                mm_i += 1

        for idx in range(NT - 1):
            rb, cw = tile_rc(idx)
            if idx not in xts:
                xt = xpool.tile([128, W], F32)
                nc.sync.dma_start(out=xt[:], in_=x[rb, :, cw * W:(cw + 1) * W])
            else:
                xt = xts[idx]
            do_tile(xt, rb, cw * W, W)

        # last tile in NSTRIP strips so the post-DMA tail chain is short
        SW = W // NSTRIP
        rb, cw = tile_rc(NT - 1)
        for s in range(NSTRIP):
            c0 = cw * W + s * SW
            xs = xpool.tile([128, SW], F32, tag="xs")
            nc.sync.dma_start(out=xs[:], in_=x[rb, :, c0:c0 + SW])
            do_tile(xs, rb, c0, SW)

        # --- combine: R[128, MMW] -> rs[128,1] -> DRAM --------------------
        rs = spool.tile([128, 1], F32)
        nc.vector.tensor_reduce(
            rs[:], R[:], axis=mybir.AxisListType.X, op=Op.add
        )
        nc.sync.dma_start(out=out[:, :], in_=rs[:])

    nc.finalize()
    return nc


_NC = None


def _get_nc() -> bass.Bass:
    global _NC
    if _NC is None:
        _NC = _build()
    return _NC


def _to_bf16(a: np.ndarray) -> np.ndarray:
    try:
        import ml_dtypes

        return a.astype(ml_dtypes.bfloat16)
    except ImportError:
        u = np.ascontiguousarray(a, dtype=np.float32).view(np.uint32)
        return ((u + 0x7FFF + ((u >> 16) & 1)) >> 16).astype(np.uint16)


def make_in_maps(input: np.ndarray, target: np.ndarray) -> list[dict]:
    x = np.ascontiguousarray(np.asarray(input, dtype=np.float32)).reshape(
        N_CORES, RB, 128, C
    )
    tf = np.asarray(target).astype(np.float64)
    n2 = C * (tf - MU) ** 2 + K
    bias = np.log(COEFF) - 0.5 * np.log(n2)
    bias = np.where(tf == 0, -1e4, bias).astype(np.float32)  # exp -> 0.0
    t2 = np.ascontiguousarray(tf.astype(np.float32).reshape(N_CORES, RB, 128, 1))
    bs = np.ascontiguousarray(bias.reshape(N_CORES, RB, 128, 1))
    ident_bf16 = _to_bf16(np.eye(128, dtype=np.float32))
    return [
        {"x": x[i], "t2": t2[i], "bs": bs[i], "ident": ident_bf16}
        for i in range(N_CORES)
    ]


def run(input: np.ndarray, target: np.ndarray, trace: bool = False, tmpdir=None):
    nc = _get_nc()
    in_maps = make_in_maps(input, target)
    res = run_bass_kernel_spmd(
        nc, in_maps, list(range(N_CORES)), trace=trace, tmpdir=tmpdir
    )
    total = np.float32(0.0)
    for r in res.results:
        total += np.float32(np.sum(np.asarray(r["out"], dtype=np.float32)))
    return np.asarray(total, dtype=np.float32), res


def kernel(input: np.ndarray, target: np.ndarray) -> np.ndarray:
    out, _ = run(input, target)
    return out


# revision 13
# speedup vs baseline: 1.1542x; 1.1542x over previous
"""DistanceFromAnswerLoss on 8 Trainium2 NeuronCores.

out = 0.1 * sum_{b,c} mask[b,c] * exp(input[b,c])
  mask[b,c] = |c - t_b| / sqrt(sum_c (c - t_b)^2),  mask = 0 where t_b == 0

Sharding: data-parallel over the batch dim (512 rows per core); each core
emits 128 partial sums + 2 accumulator columns, host adds them.

Per-core pipeline:
  host    : x cast to bf16 (halves HBM traffic; error ~1e-5 on the sum),
            bias_b = ln(0.1) - 0.5*ln(C*(t-mu)^2 + K) (-1e4 for t==0)
  DMA     : one [128,8] aux tile (4 t cols + 4 bias cols) right after the
            first x tile; per-rb contiguous [128,1] scalar tiles are made
            with on-chip copies (tiny scatter DMAs clog the rings).
  ScalarE : e' = exp(x + bias_b) -> bf16; for C_TILES also the
            abs+row-accumulate of p (balances DVE vs ScalarE).
  VectorE : ds = (iota - t_rb) + base   tensor_scalar dual-op, 4x
            dt = ds & 0x7fff (bf16 abs) tensor_scalar on uint16 view, 4x
            p  = dt * e'                tensor_tensor, 2x
  TensorE : row-reduce via identity-stationary matmuls accumulating
            R[b, j] += p[b, MMW*k + j] into PSUM; final DVE tensor_reduce.
iota [128,8192] comes from gpsimd in 2048 chunks; tile 0 splits its TS per
chunk so it never waits for later chunks (cw-major tile order).
"""

import os
import sys
from contextlib import ExitStack

import numpy as np

sys.path.insert(0, "/opt/trn_rl_repo")

import concourse.bass as bass
import concourse.tile as tile
from concourse import bacc, mybir
from concourse.bass_utils import run_bass_kernel_spmd

B = 4096
C = 8192
N_CORES = 8
ROWS = B // N_CORES          # 512 rows per core
RB = ROWS // 128             # 4 row blocks of 128 partitions
W = 4096                     # column tile width
NW = C // W
NT = RB * NW                 # 8 big tiles per core
NSTRIP = 4                   # last tile is split into NSTRIP strips
MMW = 512                    # matmul moving width (= one PSUM bank of f32)
IW = 2048                    # iota generation chunk width
X_F32 = bool(os.environ.get("X_F32"))   # A/B: keep x in f32
C_TILES = (4, 5)             # tiles reduced on ScalarE (abs+accum), not PE
COEFF = 0.1

MU = (C - 1) / 2.0
_S1 = (C - 1) * C // 2
_S2 = (C - 1) * C * (2 * C - 1) // 6
K = float(_S2 - _S1 * _S1 / C)   # sum_c (c-t)^2 = C*(t-MU)^2 + K

F32 = mybir.dt.float32
BF16 = mybir.dt.bfloat16
U16 = mybir.dt.uint16
Af = mybir.ActivationFunctionType
Op = mybir.AluOpType
XDT = F32 if X_F32 else BF16


def _build() -> bass.Bass:
    nc = bacc.Bacc("TRN2", target_bir_lowering=False, debug=False)
    x = nc.declare_dram_parameter("x", [RB, 128, C], XDT, isOutput=False)
    aux = nc.declare_dram_parameter("aux", [128, 2 * RB], F32, isOutput=False)
    ident = nc.declare_dram_parameter("ident", [128, 128], BF16, isOutput=False)
    out = nc.declare_dram_parameter("out", [128, 1 + len(C_TILES)], F32,
                                    isOutput=True)

    with tile.TileContext(nc) as tc, ExitStack() as ctx:
        const_pool = ctx.enter_context(tc.tile_pool(name="const", bufs=1))
        xpool = ctx.enter_context(tc.tile_pool(name="x", bufs=4))
        epool = ctx.enter_context(tc.tile_pool(name="e", bufs=3))
        dpool = ctx.enter_context(tc.tile_pool(name="d", bufs=2))
        ppool = ctx.enter_context(tc.tile_pool(name="p", bufs=2))
        jpool = ctx.enter_context(tc.tile_pool(name="j", bufs=2))
        spool = ctx.enter_context(tc.tile_pool(name="s", bufs=1))
        psum_pool = ctx.enter_context(tc.tile_pool(name="ps", bufs=1, space="PSUM"))

        def tile_rc(idx):
            return idx % RB, idx // RB  # rb, cw  (cw-major order)

        # --- sync queue: xt0, aux, ident, then the x stream ---------------
        xts = {}
        rb0, cw0 = tile_rc(0)
        xt0 = xpool.tile([128, W], XDT)
        nc.sync.dma_start(out=xt0[:], in_=x[rb0, :, cw0 * W:(cw0 + 1) * W])
        xts[0] = xt0
        auxt = const_pool.tile([128, 2 * RB], F32)
        nc.sync.dma_start(out=auxt[:], in_=aux[:, :])
        idt = const_pool.tile([128, 128], BF16)
        nc.sync.dma_start(out=idt[:], in_=ident[:, :])
        for idx in range(1, 3):
            rb, cw = tile_rc(idx)
            xt = xpool.tile([128, W], XDT)
            nc.sync.dma_start(out=xt[:], in_=x[rb, :, cw * W:(cw + 1) * W])
            xts[idx] = xt

        # contiguous [128,1] per-rb scalar tiles via on-chip copies
        tcols, bcols = [], []
        for rb in range(RB):
            tc_rb = const_pool.tile([128, 1], F32, tag=f"tc{rb}")
            nc.vector.tensor_copy(tc_rb[:], auxt[:, rb:rb + 1])
            tcols.append(tc_rb)
            bc_rb = const_pool.tile([128, 1], F32, tag=f"bc{rb}")
            nc.vector.tensor_copy(bc_rb[:], auxt[:, RB + rb:RB + rb + 1])
            bcols.append(bc_rb)
        absmask = spool.tile([128, 1], U16)
        nc.vector.memset(absmask[:], 0x7FFF)

        iota = const_pool.tile([128, C], BF16)
        for ci in range(C // IW):
            nc.gpsimd.iota(
                iota[:, ci * IW:(ci + 1) * IW],
                pattern=[[1, IW]], base=ci * IW, channel_multiplier=0,
                allow_small_or_imprecise_dtypes=True,
            )

        # --- main loop ----------------------------------------------------
        R = psum_pool.tile([128, MMW], F32)
        acc = spool.tile([128, max(1, len(C_TILES))], F32)
        pe_tiles = [i for i in range(NT) if i not in C_TILES]
        n_mm = sum(W // MMW for i in pe_tiles if i < NT - 1)
        if NT - 1 in pe_tiles:
            n_mm += NSTRIP * (W // NSTRIP // MMW)
        mm_i = 0

        def do_tile(xt, idx, rb, c0, width, split_ts):
            nonlocal mm_i
            big = width == W
            et = epool.tile([128, width], BF16, tag="e" if big else "es")
            nc.scalar.activation(et[:], xt[:], Af.Exp, bias=bcols[rb][:])
            ds = dpool.tile([128, width], BF16, tag="d0" if big else "d0s")
            step = IW if split_ts else width
            for o in range(0, width, step):
                bw = min(step, width - o)
                nc.vector.tensor_scalar(
                    ds[:, o:o + bw], iota[:, c0 + o:c0 + o + bw], tcols[rb][:],
                    None, op0=Op.subtract,
                )
            pe = idx in pe_tiles
            if pe:
                dt = dpool.tile([128, width], BF16, tag="d" if big else "dss")
                mask_arg = 0x7FFF if os.environ.get("ABS_IMM") else absmask[:]
                nc.vector.tensor_scalar(
                    dt[:].bitcast(U16), ds[:].bitcast(U16), mask_arg, None,
                    op0=Op.bitwise_and,
                )
            else:
                dt = ds
            pt = ppool.tile([128, width], BF16, tag="p" if big else "ps")
            nc.vector.tensor_tensor(pt[:], dt[:], et[:], op=Op.mult)
            if pe:
                for j in range(width // MMW):
                    nc.tensor.matmul(
                        R[:], idt[:], pt[:, j * MMW:(j + 1) * MMW],
                        start=(mm_i == 0), stop=(mm_i == n_mm - 1),
                    )
                    mm_i += 1
            else:
                jt = jpool.tile([128, width], BF16, tag="j")
                k = C_TILES.index(idx)
                nc.scalar.activation(
                    jt[:], pt[:], Af.Abs, accum_out=acc[:, k:k + 1]
                )

        for idx in range(NT - 1):
            rb, cw = tile_rc(idx)
            if idx not in xts:
                xt = xpool.tile([128, W], XDT)
                nc.sync.dma_start(out=xt[:], in_=x[rb, :, cw * W:(cw + 1) * W])
            else:
                xt = xts[idx]
            do_tile(xt, idx, rb, cw * W, W, split_ts=(idx == 0))

        SW = W // NSTRIP
        rb, cw = tile_rc(NT - 1)
        for s in range(NSTRIP):
            c0 = cw * W + s * SW
            xs = xpool.tile([128, SW], XDT, tag="xs")
            nc.sync.dma_start(out=xs[:], in_=x[rb, :, c0:c0 + SW])
            do_tile(xs, NT - 1, rb, c0, SW, split_ts=False)

        # --- combine: rs = rowsum(R); out = [rs, acc] ---------------------
        rs = spool.tile([128, 1 + len(C_TILES)], F32)
        nc.vector.tensor_reduce(
            rs[:, 0:1], R[:], axis=mybir.AxisListType.X, op=Op.add
        )
        if C_TILES:
            nc.vector.tensor_copy(rs[:, 1:], acc[:])
        nc.sync.dma_start(out=out[:, :], in_=rs[:])

    nc.finalize()
    return nc


_NC = None


def _get_nc() -> bass.Bass:
    global _NC
    if _NC is None:
        _NC = _build()
    return _NC


def _to_bf16(a: np.ndarray) -> np.ndarray:
    import ml_dtypes

    return a.astype(ml_dtypes.bfloat16)


def make_in_maps(input: np.ndarray, target: np.ndarray) -> list[dict]:
    x = np.ascontiguousarray(np.asarray(input, dtype=np.float32)).reshape(
        N_CORES, RB, 128, C
    )
    if not X_F32:
        x = _to_bf16(x)
    tf = np.asarray(target).astype(np.float64)
    n2 = C * (tf - MU) ** 2 + K
    bias = np.log(COEFF) - 0.5 * np.log(n2)
    bias = np.where(tf == 0, -1e4, bias).astype(np.float32)  # exp -> 0.0
    tv = tf.astype(np.float32).reshape(N_CORES, RB, 128)
    bv = bias.reshape(N_CORES, RB, 128)
    # aux[p, 0:RB] = t per row block, aux[p, RB:2RB] = bias per row block
    aux = np.concatenate(
        [tv.transpose(0, 2, 1), bv.transpose(0, 2, 1)], axis=2
    )
    aux = np.ascontiguousarray(aux, dtype=np.float32)
    ident_bf16 = _to_bf16(np.eye(128, dtype=np.float32))
    return [
        {"x": x[i], "aux": aux[i], "ident": ident_bf16}
        for i in range(N_CORES)
    ]


def run(input: np.ndarray, target: np.ndarray, trace: bool = False, tmpdir=None):
    nc = _get_nc()
    in_maps = make_in_maps(input, target)
    res = run_bass_kernel_spmd(
        nc, in_maps, list(range(N_CORES)), trace=trace, tmpdir=tmpdir
    )
    total = np.float32(0.0)
    for r in res.results:
        total += np.float32(np.sum(np.asarray(r["out"], dtype=np.float32)))
    return np.asarray(total, dtype=np.float32), res


def kernel(input: np.ndarray, target: np.ndarray) -> np.ndarray:
    out, _ = run(input, target)
    return out


# revision 15
# speedup vs baseline: 1.4008x; 1.2136x over previous
"""DistanceFromAnswerLoss on 8 Trainium2 NeuronCores.

out = 0.1 * sum_{b,c} mask[b,c] * exp(input[b,c])
  mask[b,c] = |c - t_b| / sqrt(sum_c (c - t_b)^2),  mask = 0 where t_b == 0

Sharding: data-parallel over the batch dim (512 rows per core); each core
emits one f32 scalar, host adds the 8.

Per-core pipeline:
  host    : x cast to bf16 (halves HBM traffic; error ~1e-5 on the sum),
            bias_b = ln(0.1) - 0.5*ln(C*(t-mu)^2 + K) (-1e4 for t==0)
  ScalarE : e' = exp(x + bias_b) -> bf16 (only pass; one table load)
  VectorE : ds = iota - t_rb             tensor_scalar, 4x
            dt = ds & 0x7fff (bf16 abs)  tensor_scalar on uint16 view, 4x
            p  = dt * e'                 tensor_tensor, 2x
  TensorE : row-reduce via identity-stationary matmuls accumulating
            R[b, j] += p[b, 512k + j] into one PSUM bank; final DVE
            tensor_reduce + ones-matmul -> [1,1] scalar out.
iota [128,8192]: gpsimd writes only [0:2048] (concurrent gpsimd writes
throttle DVE reads ~3x), DVE extends it with two 4x tensor_scalar adds.
Per-row scalars arrive in one [128, 2*RB] aux DMA and are copied on-chip
to contiguous [128,1] tiles.  cw-major tile order; first x DMA leads the
sync queue.
"""

import os
import sys
from contextlib import ExitStack

import numpy as np

sys.path.insert(0, "/opt/trn_rl_repo")

import concourse.bass as bass
import concourse.tile as tile
from concourse import bacc, mybir
from concourse.bass_utils import run_bass_kernel_spmd

B = 4096
C = 8192
N_CORES = 8
ROWS = B // N_CORES          # 512 rows per core
RB = ROWS // 128             # 4 row blocks of 128 partitions
W = 4096                     # column tile width
NW = C // W
NT = RB * NW                 # 8 big tiles per core
NSTRIP = 4                   # last tile is split into NSTRIP strips
MMW = 512                    # matmul moving width (= one PSUM bank of f32)
IW = 2048                    # gpsimd iota width; DVE extends to C
X_F32 = bool(os.environ.get("X_F32"))   # A/B: keep x in f32
COEFF = 0.1

MU = (C - 1) / 2.0
_S1 = (C - 1) * C // 2
_S2 = (C - 1) * C * (2 * C - 1) // 6
K = float(_S2 - _S1 * _S1 / C)   # sum_c (c-t)^2 = C*(t-MU)^2 + K

F32 = mybir.dt.float32
BF16 = mybir.dt.bfloat16
U16 = mybir.dt.uint16
Af = mybir.ActivationFunctionType
Op = mybir.AluOpType
XDT = F32 if X_F32 else BF16


def _build() -> bass.Bass:
    nc = bacc.Bacc("TRN2", target_bir_lowering=False, debug=False)
    x = nc.declare_dram_parameter("x", [RB, 128, C], XDT, isOutput=False)
    aux = nc.declare_dram_parameter("aux", [128, 2 * RB], F32, isOutput=False)
    ident = nc.declare_dram_parameter("ident", [128, 128], BF16, isOutput=False)
    out = nc.declare_dram_parameter("out", [1, 1], F32, isOutput=True)

    with tile.TileContext(nc) as tc, ExitStack() as ctx:
        const_pool = ctx.enter_context(tc.tile_pool(name="const", bufs=1))
        xpool = ctx.enter_context(tc.tile_pool(name="x", bufs=5))
        epool = ctx.enter_context(tc.tile_pool(name="e", bufs=4))
        dpool = ctx.enter_context(tc.tile_pool(name="d", bufs=2))
        ppool = ctx.enter_context(tc.tile_pool(name="p", bufs=2))
        spool = ctx.enter_context(tc.tile_pool(name="s", bufs=1))
        psum_pool = ctx.enter_context(tc.tile_pool(name="ps", bufs=1, space="PSUM"))

        def tile_rc(idx):
            return idx % RB, idx // RB  # rb, cw  (cw-major order)

        # --- sync queue: xt0, aux, ident, then the x stream ---------------
        xts = {}
        rb0, cw0 = tile_rc(0)
        xt0 = xpool.tile([128, W], XDT)
        nc.sync.dma_start(out=xt0[:], in_=x[rb0, :, cw0 * W:(cw0 + 1) * W])
        xts[0] = xt0
        auxt = const_pool.tile([128, 2 * RB], F32)
        nc.sync.dma_start(out=auxt[:], in_=aux[:, :])
        idt = const_pool.tile([128, 128], BF16)
        nc.sync.dma_start(out=idt[:], in_=ident[:, :])
        for idx in range(1, 3):
            rb, cw = tile_rc(idx)
            xt = xpool.tile([128, W], XDT)
            nc.sync.dma_start(out=xt[:], in_=x[rb, :, cw * W:(cw + 1) * W])
            xts[idx] = xt

        # contiguous [128,1] per-rb scalar tiles via on-chip copies
        tcols, bcols = [], []
        for rb in range(RB):
            tc_rb = const_pool.tile([128, 1], F32, tag=f"tc{rb}")
            nc.vector.tensor_copy(tc_rb[:], auxt[:, rb:rb + 1])
            tcols.append(tc_rb)
            bc_rb = const_pool.tile([128, 1], F32, tag=f"bc{rb}")
            nc.vector.tensor_copy(bc_rb[:], auxt[:, RB + rb:RB + rb + 1])
            bcols.append(bc_rb)
        absmask = spool.tile([128, 1], U16)
        nc.vector.memset(absmask[:], 0x7FFF)
        ones = const_pool.tile([128, 1], F32)
        nc.vector.memset(ones[:], 1.0)

        # iota: gpsimd seed [0:IW], then DVE doubles it twice (4x ts adds)
        iota = const_pool.tile([128, C], BF16)
        nc.gpsimd.iota(
            iota[:, 0:IW], pattern=[[1, IW]], base=0, channel_multiplier=0,
            allow_small_or_imprecise_dtypes=True,
        )
        nc.vector.tensor_scalar(
            iota[:, IW:2 * IW], iota[:, 0:IW], float(IW), None, op0=Op.add
        )
        nc.vector.tensor_scalar(
            iota[:, 2 * IW:C], iota[:, 0:2 * IW], float(2 * IW), None,
            op0=Op.add,
        )

        # --- main loop ----------------------------------------------------
        R = psum_pool.tile([128, MMW], F32)
        n_mm = (NT - 1) * (W // MMW) + NSTRIP * (W // NSTRIP // MMW)
        mm_i = 0

        def do_tile(xt, rb, c0, width):
            nonlocal mm_i
            big = width == W
            et = epool.tile([128, width], BF16, tag="e" if big else "es")
            nc.scalar.activation(et[:], xt[:], Af.Exp, bias=bcols[rb][:])
            ds = dpool.tile([128, width], BF16, tag="d0" if big else "d0s")
            nc.vector.tensor_scalar(
                ds[:], iota[:, c0:c0 + width], tcols[rb][:], None,
                op0=Op.subtract,
            )
            dt = dpool.tile([128, width], BF16, tag="d" if big else "dss")
            mask_arg = 0x7FFF if os.environ.get("ABS_IMM") else absmask[:]
            nc.vector.tensor_scalar(
                dt[:].bitcast(U16), ds[:].bitcast(U16), mask_arg, None,
                op0=Op.bitwise_and,
            )
            pt = ppool.tile([128, width], BF16, tag="p" if big else "ps")
            nc.vector.tensor_tensor(pt[:], dt[:], et[:], op=Op.mult)
            for j in range(width // MMW):
                nc.tensor.matmul(
                    R[:], idt[:], pt[:, j * MMW:(j + 1) * MMW],
                    start=(mm_i == 0), stop=(mm_i == n_mm - 1),
                )
                mm_i += 1

        for idx in range(NT - 1):
            rb, cw = tile_rc(idx)
            if idx not in xts:
                xt = xpool.tile([128, W], XDT)
                nc.sync.dma_start(out=xt[:], in_=x[rb, :, cw * W:(cw + 1) * W])
            else:
                xt = xts[idx]
            do_tile(xt, rb, cw * W, W)

        SW = W // NSTRIP
        rb, cw = tile_rc(NT - 1)
        for s in range(NSTRIP):
            c0 = cw * W + s * SW
            xs = xpool.tile([128, SW], XDT, tag="xs")
            nc.sync.dma_start(out=xs[:], in_=x[rb, :, c0:c0 + SW])
            do_tile(xs, rb, c0, SW)

        # --- combine: rs = rowsum(R); tot = ones . rs -> [1,1] ------------
        rs = spool.tile([128, 1], F32)
        nc.vector.tensor_reduce(
            rs[:], R[:], axis=mybir.AxisListType.X, op=Op.add
        )
        tot_ps = psum_pool.tile([1, 1], F32)
        nc.tensor.matmul(tot_ps[:], ones[:], rs[:], start=True, stop=True)
        tot = spool.tile([1, 1], F32)
        nc.vector.tensor_copy(tot[:], tot_ps[:])
        nc.sync.dma_start(out=out[:, :], in_=tot[:])

    nc.finalize()
    return nc


_NC = None


def _get_nc() -> bass.Bass:
    global _NC
    if _NC is None:
        _NC = _build()
    return _NC


def _to_bf16(a: np.ndarray) -> np.ndarray:
    import ml_dtypes

    return a.astype(ml_dtypes.bfloat16)


def make_in_maps(input: np.ndarray, target: np.ndarray) -> list[dict]:
    x = np.ascontiguousarray(np.asarray(input, dtype=np.float32)).reshape(
        N_CORES, RB, 128, C
    )
    if not X_F32:
        x = _to_bf16(x)
    tf = np.asarray(target).astype(np.float64)
    n2 = C * (tf - MU) ** 2 + K
    bias = np.log(COEFF) - 0.5 * np.log(n2)
    bias = np.where(tf == 0, -1e4, bias).astype(np.float32)  # exp -> 0.0
    tv = tf.astype(np.float32).reshape(N_CORES, RB, 128)
    bv = bias.reshape(N_CORES, RB, 128)
    aux = np.concatenate(
        [tv.transpose(0, 2, 1), bv.transpose(0, 2, 1)], axis=2
    )
    aux = np.ascontiguousarray(aux, dtype=np.float32)
    ident_bf16 = _to_bf16(np.eye(128, dtype=np.float32))
    return [
        {"x": x[i], "aux": aux[i], "ident": ident_bf16} for i in range(N_CORES)
    ]


def run(input: np.ndarray, target: np.ndarray, trace: bool = False, tmpdir=None):
    nc = _get_nc()
    in_maps = make_in_maps(input, target)
    res = run_bass_kernel_spmd(
        nc, in_maps, list(range(N_CORES)), trace=trace, tmpdir=tmpdir
    )
    total = np.float32(0.0)
    for r in res.results:
        total += np.float32(np.sum(np.asarray(r["out"], dtype=np.float32)))
    return np.asarray(total, dtype=np.float32), res


def kernel(input: np.ndarray, target: np.ndarray) -> np.ndarray:
    out, _ = run(input, target)
    return out


# revision 16
# speedup vs baseline: 1.4065x; 1.0041x over previous
"""DistanceFromAnswerLoss on 8 Trainium2 NeuronCores — fused custom-DVE op, v2.

out = 0.1 * sum_{b,c} mask[b,c] * exp(input[b,c])
  mask[b,c] = |c - t_b| / sqrt(sum_c (c - t_b)^2),  mask = 0 where t_b == 0

Per-core pipeline:
  host    : x cast to bf16; bias_b = ln(0.1) - 0.5*ln(C*(t-mu)^2 + K)
            (-1e4 for t==0); aux also carries t-2048 and t-4096 so every
            fused call indexes the same iota[0:4096] window.
  ScalarE : a dummy [128,1] exp FIRST in program order so the activation
            table loads before the x stream saturates the DMA rings;
            then e' = exp(x + bias_b) -> bf16 per tile.
  VectorE : ONE fused op per tile: out = max(iota - t', t' - iota) * e',
            accum_out = rowsum(out)  — |c-t|*e' and the row reduction in
            a single pass.  Tile 0 is split 2x2048 so the chain starts as
            soon as the gpsimd iota seed lands; the only iota expansion
            ([2048:4096], one 4x tensor_scalar) hides between fused calls.
  TensorE : ones-matmul for the final partition reduce -> [1,1] scalar.
"""

import os
import sys
from contextlib import ExitStack

import numpy as np

sys.path.insert(0, "/opt/trn_rl_repo")

import concourse.bass as bass
import concourse.tile as tile
from concourse import bacc, mybir
from concourse.bass_utils import run_bass_kernel_spmd

B = 4096
C = 8192
N_CORES = 8
ROWS = B // N_CORES
RB = ROWS // 128
W = 4096
NW = C // W
NT = RB * NW                 # 8 tile-equivalents per core
IW = 2048                    # gpsimd iota seed width; expanded once to 2*IW
NACC = NT + 1                # tile 0 split into two accumulator columns
X_F32 = bool(os.environ.get("X_F32"))
COEFF = 0.1

MU = (C - 1) / 2.0
_S1 = (C - 1) * C // 2
_S2 = (C - 1) * C * (2 * C - 1) // 6
K = float(_S2 - _S1 * _S1 / C)

F32 = mybir.dt.float32
BF16 = mybir.dt.bfloat16
Af = mybir.ActivationFunctionType
Op = mybir.AluOpType
XDT = F32 if X_F32 else BF16

_OP_NAME = "ABSDIFF_MUL_REDUCE_ANT"


def _register_dist_op():
    """out = max(Src0 - s0, s0 - Src0) * Src1 ; accum_out = rowsum(out)."""
    from operator import add

    from concourse import dve_ops
    from concourse.dve_spec import C0, Spec, Src0, Src1, lower, maxx
    from concourse.dve_uop import DveOpSpec

    for op in dve_ops.OPS:
        if op.name == _OP_NAME:
            return op

    def _ref(in0, in1, s0, s1, imm2):
        s = np.asarray(s0, dtype=np.float64)
        d = np.abs(in0.astype(np.float64) - s)
        b = (d * in1.astype(np.float64)).astype(np.float32)
        acc = (
            b.reshape(b.shape[0], -1).astype(np.float64).sum(-1, keepdims=True)
        ).astype(np.float32)
        return b, acc

    spec = Spec(
        body=maxx(Src0 - C0, C0 - Src0) * Src1, accum=add,
        reference=_ref,
    )
    row = dve_ops._CUSTOM_DVE_ROW_BASE + len(dve_ops.OPS)
    shas = {
        ver: DveOpSpec(
            name=_OP_NAME, opcode=row, uops=lower(spec, ver=ver), rd1_en=True
        ).sha(ver)
        for ver in ("v3", "v4")
    }
    op = dve_ops.DveOp(_OP_NAME, spec, subdim=False, uops_sha=shas)
    dve_ops.OPS.append(op)
    dve_ops.CUSTOM_DVE_SPECS[op.name] = spec
    dve_ops._SUB_OPCODE_FOR_NAME[op.name] = row
    return op


# schedule: (rb, c0, width, scalar-group)   g: 0 -> t, 1 -> t-2048, 2 -> t-4096
def _schedule():
    s = [(0, 0, IW, 0), (0, IW, IW, 1)]
    for idx in range(1, NT):
        rb, cw = idx % RB, idx // RB
        s.append((rb, cw * W, W, 2 * cw))
    return s


def _build() -> bass.Bass:
    dist_op = _register_dist_op()
    nc = bacc.Bacc("TRN2", target_bir_lowering=False, debug=False)
    x = nc.declare_dram_parameter("x", [RB, 128, C], XDT, isOutput=False)
    # aux columns: [t(4) | t-2048(4) | t-4096(4) | bias(4)]
    aux = nc.declare_dram_parameter("aux", [128, 4 * RB], F32, isOutput=False)
    out = nc.declare_dram_parameter("out", [1, 1], F32, isOutput=True)

    sched = _schedule()

    with tile.TileContext(nc) as tc, ExitStack() as ctx:
        const_pool = ctx.enter_context(tc.tile_pool(name="const", bufs=1))
        xpool = ctx.enter_context(tc.tile_pool(name="x", bufs=5))
        epool = ctx.enter_context(tc.tile_pool(name="e", bufs=4))
        dpool = ctx.enter_context(tc.tile_pool(name="d", bufs=2))
        spool = ctx.enter_context(tc.tile_pool(name="s", bufs=1))
        psum_pool = ctx.enter_context(tc.tile_pool(name="ps", bufs=1, space="PSUM"))

        # --- activation-table preload: dummy exp BEFORE any DMA -----------
        dum0 = const_pool.tile([128, 1], F32)
        nc.vector.memset(dum0[:], 0.0)
        dum1 = const_pool.tile([128, 1], BF16)
        nc.scalar.activation(dum1[:], dum0[:], Af.Exp)

        # --- sync queue: xt0a, aux, then the x stream ---------------------
        xts = {}
        rb0, c00, w0, _ = sched[0]
        xt = xpool.tile([128, w0], XDT, tag="xs")
        nc.sync.dma_start(out=xt[:], in_=x[rb0, :, c00:c00 + w0])
        xts[0] = xt
        auxt = const_pool.tile([128, 4 * RB], F32)
        nc.sync.dma_start(out=auxt[:], in_=aux[:, :])
        for si in range(1, 3):
            rb, c0, wd, _ = sched[si]
            xt = xpool.tile([128, wd], XDT, tag="xs" if wd == IW else None)
            nc.sync.dma_start(out=xt[:], in_=x[rb, :, c0:c0 + wd])
            xts[si] = xt

        # bias tiles are copied on the (otherwise idle) ScalarE queue so the
        # DVE copy burst does not gate the first exp: bcol0 right before
        # exp0, the rest after exp1 (sched[0..1] are both rb0)
        bcols = []
        for rb in range(RB):
            bc = const_pool.tile([128, 1], F32, tag=f"bc{rb}")
            bcols.append(bc)
        nc.scalar.copy(bcols[0][:], auxt[:, 3 * RB:3 * RB + 1])
        ones = const_pool.tile([128, 1], F32)
        nc.vector.memset(ones[:], 1.0)
        # (g, rb) -> tile; only the used ones are copied
        used = sorted({(g, rb) for rb, _, _, g in sched})
        tg = {}
        for g, rb in used:
            t_ = const_pool.tile([128, 1], F32, tag=f"t{g}_{rb}")
            nc.vector.tensor_copy(t_[:], auxt[:, g * RB + rb:g * RB + rb + 1])
            tg[(g, rb)] = t_

        acc = spool.tile([128, NACC], F32)

        iota = const_pool.tile([128, 2 * IW], BF16)
        nc.gpsimd.iota(
            iota[:, 0:IW], pattern=[[1, IW]], base=0, channel_multiplier=0,
            allow_small_or_imprecise_dtypes=True,
        )

        def fused(si, et, rb, width, g):
            dm = dpool.tile([128, width], BF16, tag="dm" if width == W else "dms")
            nc.vector._custom_dve(
                dist_op, out=dm[:], in0=iota[:, 0:width], in1=et[:],
                s0=tg[(g, rb)][:], accum_out=acc[:, si:si + 1],
            )

        ets = {}
        for si, (rb, c0, wd, g) in enumerate(sched):
            if si not in xts:
                xt = xpool.tile([128, wd], XDT)
                nc.sync.dma_start(out=xt[:], in_=x[rb, :, c0:c0 + wd])
            else:
                xt = xts[si]
            et = epool.tile([128, wd], BF16, tag="es" if wd == IW else "e")
            nc.scalar.activation(et[:], xt[:], Af.Exp, bias=bcols[rb][:])
            ets[si] = et
            fused(si, et, rb, wd, g)
            if si == 1:
                # iota[2048:4096] = iota[0:2048] + 2048 — hidden between
                # fused calls; first needed by sched[2]
                nc.vector.tensor_scalar(
                    iota[:, IW:2 * IW], iota[:, 0:IW], float(IW), None,
                    op0=Op.add,
                )
                for rb in range(1, RB):
                    nc.scalar.copy(
                        bcols[rb][:], auxt[:, 3 * RB + rb:3 * RB + rb + 1]
                    )

        # --- combine: rs = rowsum(acc); tot = ones . rs -> [1,1] ----------
        rs = spool.tile([128, 1], F32)
        nc.vector.tensor_reduce(
            rs[:], acc[:], axis=mybir.AxisListType.X, op=Op.add
        )
        tot_ps = psum_pool.tile([1, 1], F32)
        nc.tensor.matmul(tot_ps[:], ones[:], rs[:], start=True, stop=True)
        tot = spool.tile([1, 1], F32)
        nc.vector.tensor_copy(tot[:], tot_ps[:])
        nc.sync.dma_start(out=out[:, :], in_=tot[:])

    nc.finalize()
    return nc


_NC = None


def _get_nc() -> bass.Bass:
    global _NC
    if _NC is None:
        _NC = _build()
    return _NC


def _to_bf16(a: np.ndarray) -> np.ndarray:
    import ml_dtypes

    return a.astype(ml_dtypes.bfloat16)


def make_in_maps(input: np.ndarray, target: np.ndarray) -> list[dict]:
    x = np.ascontiguousarray(np.asarray(input, dtype=np.float32)).reshape(
        N_CORES, RB, 128, C
    )
    if not X_F32:
        x = _to_bf16(x)
    tf = np.asarray(target).astype(np.float64)
    n2 = C * (tf - MU) ** 2 + K
    bias = np.log(COEFF) - 0.5 * np.log(n2)
    bias = np.where(tf == 0, -1e4, bias).astype(np.float32)
    tv = tf.astype(np.float32).reshape(N_CORES, RB, 128)
    bv = bias.reshape(N_CORES, RB, 128)
    cols = [tv, tv - 2048.0, tv - 4096.0, bv]
    aux = np.concatenate([c.transpose(0, 2, 1) for c in cols], axis=2)
    aux = np.ascontiguousarray(aux, dtype=np.float32)
    return [{"x": x[i], "aux": aux[i]} for i in range(N_CORES)]


def run(input: np.ndarray, target: np.ndarray, trace: bool = False, tmpdir=None):
    nc = _get_nc()
    in_maps = make_in_maps(input, target)
    res = run_bass_kernel_spmd(
        nc, in_maps, list(range(N_CORES)), trace=trace, tmpdir=tmpdir
    )
    total = np.float32(0.0)
    for r in res.results:
        total += np.float32(np.sum(np.asarray(r["out"], dtype=np.float32)))
    return np.asarray(total, dtype=np.float32), res


def kernel(input: np.ndarray, target: np.ndarray) -> np.ndarray:
    out, _ = run(input, target)
    return out


# revision 18
# speedup vs baseline: 1.4177x; 1.0080x over previous
"""DistanceFromAnswerLoss on 8 Trainium2 NeuronCores — fused custom-DVE op, v2.

out = 0.1 * sum_{b,c} mask[b,c] * exp(input[b,c])
  mask[b,c] = |c - t_b| / sqrt(sum_c (c - t_b)^2),  mask = 0 where t_b == 0

Per-core pipeline:
  host    : x cast to bf16; bias_b = ln(0.1) - 0.5*ln(C*(t-mu)^2 + K)
            (-1e4 for t==0); aux also carries t-2048 and t-4096 so every
            fused call indexes the same iota[0:4096] window.
  ScalarE : a dummy [128,1] exp FIRST in program order so the activation
            table loads before the x stream saturates the DMA rings;
            then e' = exp(x + bias_b) -> bf16 per tile.
  VectorE : ONE fused op per tile: out = max(iota - t', t' - iota) * e',
            accum_out = rowsum(out)  — |c-t|*e' and the row reduction in
            a single pass.  Tile 0 is split 2x2048 so the chain starts as
            soon as the gpsimd iota seed lands; the only iota expansion
            ([2048:4096], one 4x tensor_scalar) hides between fused calls.
  TensorE : ones-matmul for the final partition reduce -> [1,1] scalar.
"""

import os
import sys
from contextlib import ExitStack

import numpy as np

sys.path.insert(0, "/opt/trn_rl_repo")

import concourse.bass as bass
import concourse.tile as tile
from concourse import bacc, mybir
from concourse.bass_utils import run_bass_kernel_spmd

B = 4096
C = 8192
N_CORES = 8
ROWS = B // N_CORES
RB = ROWS // 128
W = 4096
NW = C // W
NT = RB * NW                 # 8 tile-equivalents per core
IW = 2048                    # gpsimd iota seed width; expanded once to 2*IW
NACC = NT + 1                # tile 0 split into two accumulator columns
X_F32 = bool(os.environ.get("X_F32"))
COEFF = 0.1

MU = (C - 1) / 2.0
_S1 = (C - 1) * C // 2
_S2 = (C - 1) * C * (2 * C - 1) // 6
K = float(_S2 - _S1 * _S1 / C)

F32 = mybir.dt.float32
BF16 = mybir.dt.bfloat16
Af = mybir.ActivationFunctionType
Op = mybir.AluOpType
XDT = F32 if X_F32 else BF16

_OP_NAME = "ABSDIFF_MUL_REDUCE_ANT"


def _register_dist_op():
    """out = max(Src0 - s0, s0 - Src0) * Src1 ; accum_out = rowsum(out)."""
    from operator import add

    from concourse import dve_ops
    from concourse.dve_spec import C0, Spec, Src0, Src1, lower, maxx
    from concourse.dve_uop import DveOpSpec

    for op in dve_ops.OPS:
        if op.name == _OP_NAME:
            return op

    def _ref(in0, in1, s0, s1, imm2):
        s = np.asarray(s0, dtype=np.float64)
        d = np.abs(in0.astype(np.float64) - s)
        b = (d * in1.astype(np.float64)).astype(np.float32)
        acc = (
            b.reshape(b.shape[0], -1).astype(np.float64).sum(-1, keepdims=True)
        ).astype(np.float32)
        return b, acc

    spec = Spec(
        body=maxx(Src0 - C0, C0 - Src0) * Src1, accum=add,
        reference=_ref,
    )
    row = dve_ops._CUSTOM_DVE_ROW_BASE + len(dve_ops.OPS)
    shas = {
        ver: DveOpSpec(
            name=_OP_NAME, opcode=row, uops=lower(spec, ver=ver), rd1_en=True
        ).sha(ver)
        for ver in ("v3", "v4")
    }
    op = dve_ops.DveOp(_OP_NAME, spec, subdim=False, uops_sha=shas)
    dve_ops.OPS.append(op)
    dve_ops.CUSTOM_DVE_SPECS[op.name] = spec
    dve_ops._SUB_OPCODE_FOR_NAME[op.name] = row
    return op


# schedule: (rb, c0, width, scalar-group)   g: 0 -> t, 1 -> t-2048, 2 -> t-4096
def _schedule():
    s = [(0, 0, IW, 0), (0, IW, IW, 1)]
    for idx in range(1, NT):
        rb, cw = idx % RB, idx // RB
        s.append((rb, cw * W, W, 2 * cw))
    return s


def _build() -> bass.Bass:
    dist_op = _register_dist_op()
    nc = bacc.Bacc("TRN2", target_bir_lowering=False, debug=False)
    x = nc.declare_dram_parameter("x", [RB, 128, C], XDT, isOutput=False)
    # aux columns: [t(4) | t-2048(4) | t-4096(4) | bias(4)]
    aux = nc.declare_dram_parameter("aux", [128, 4 * RB], F32, isOutput=False)
    out = nc.declare_dram_parameter("out", [1, 1], F32, isOutput=True)

    sched = _schedule()

    with tile.TileContext(nc) as tc, ExitStack() as ctx:
        const_pool = ctx.enter_context(tc.tile_pool(name="const", bufs=1))
        xpool = ctx.enter_context(tc.tile_pool(name="x", bufs=5))
        epool = ctx.enter_context(tc.tile_pool(name="e", bufs=4))
        dpool = ctx.enter_context(tc.tile_pool(name="d", bufs=2))
        spool = ctx.enter_context(tc.tile_pool(name="s", bufs=1))
        psum_pool = ctx.enter_context(tc.tile_pool(name="ps", bufs=1, space="PSUM"))

        # --- activation-table preload: dummy exp BEFORE any DMA -----------
        dum0 = const_pool.tile([128, 1], F32)
        nc.vector.memset(dum0[:], 0.0)
        dum1 = const_pool.tile([128, 1], BF16)
        nc.scalar.activation(dum1[:], dum0[:], Af.Exp)

        # --- sync queue: aux first (it gates bcol0 -> exp0), then x -------
        auxt = const_pool.tile([128, 4 * RB], F32)
        nc.sync.dma_start(out=auxt[:], in_=aux[:, :])
        xts = {}
        for si in range(3):
            rb, c0, wd, _ = sched[si]
            xt = xpool.tile([128, wd], XDT, tag="xs" if wd == IW else None)
            nc.sync.dma_start(out=xt[:], in_=x[rb, :, c0:c0 + wd])
            xts[si] = xt

        # ALL per-row scalar tiles are copied on the (otherwise idle)
        # ScalarE queue: the ones the head of the pipeline needs right
        # away here, the rest interleaved between exps in the main loop
        # (each pair costs ~0.6us of Sc slack, never pacing the chain).
        bcols, tg = [], {}
        for rb in range(RB):
            bc = const_pool.tile([128, 1], F32, tag=f"bc{rb}")
            bcols.append(bc)
        used = sorted({(g, rb) for rb, _, _, g in sched})
        for g, rb in used:
            t_ = const_pool.tile([128, 1], F32, tag=f"t{g}_{rb}")
            tg[(g, rb)] = t_

        def sc_copy_bcol(rb):
            nc.scalar.copy(bcols[rb][:], auxt[:, 3 * RB + rb:3 * RB + rb + 1])

        def sc_copy_tg(g, rb):
            nc.scalar.copy(tg[(g, rb)][:], auxt[:, g * RB + rb:g * RB + rb + 1])

        sc_copy_bcol(0)
        sc_copy_tg(0, 0)
        sc_copy_tg(1, 0)
        # remaining copies, scheduled after exp_k in the main loop:
        #   si=1: bcol1+tg(0,1); si=2: bcol2+tg(0,2); si=3: bcol3+tg(0,3)
        #   si=4..7: tg(2, 0..3)
        deferred = {
            1: [("b", 1), ("t", 0, 1)],
            2: [("b", 2), ("t", 0, 2)],
            3: [("b", 3), ("t", 0, 3)],
            4: [("t", 2, 0)],
            5: [("t", 2, 1)],
            6: [("t", 2, 2)],
            7: [("t", 2, 3)],
        }
        ones = const_pool.tile([128, 1], F32)
        nc.vector.memset(ones[:], 1.0)

        acc = spool.tile([128, NACC], F32)

        iota = const_pool.tile([128, 2 * IW], BF16)
        nc.gpsimd.iota(
            iota[:, 0:IW], pattern=[[1, IW]], base=0, channel_multiplier=0,
            allow_small_or_imprecise_dtypes=True,
        )

        def fused(si, et, rb, width, g):
            dm = dpool.tile([128, width], BF16, tag="dm" if width == W else "dms")
            nc.vector._custom_dve(
                dist_op, out=dm[:], in0=iota[:, 0:width], in1=et[:],
                s0=tg[(g, rb)][:], accum_out=acc[:, si:si + 1],
            )

        ets = {}
        for si, (rb, c0, wd, g) in enumerate(sched):
            if si not in xts:
                xt = xpool.tile([128, wd], XDT)
                nc.sync.dma_start(out=xt[:], in_=x[rb, :, c0:c0 + wd])
            else:
                xt = xts[si]
            et = epool.tile([128, wd], BF16, tag="es" if wd == IW else "e")
            nc.scalar.activation(et[:], xt[:], Af.Exp, bias=bcols[rb][:])
            ets[si] = et
            fused(si, et, rb, wd, g)
            for item in deferred.get(si, ()):
                if item[0] == "b":
                    sc_copy_bcol(item[1])
                else:
                    sc_copy_tg(item[1], item[2])
            if si == 1:
                # iota[2048:4096] = iota[0:2048] + 2048 — hidden between
                # fused calls; first needed by sched[2]
                nc.vector.tensor_scalar(
                    iota[:, IW:2 * IW], iota[:, 0:IW], float(IW), None,
                    op0=Op.add,
                )

        # --- combine: rs = rowsum(acc); tot = ones . rs -> [1,1] ----------
        rs = spool.tile([128, 1], F32)
        nc.vector.tensor_reduce(
            rs[:], acc[:], axis=mybir.AxisListType.X, op=Op.add
        )
        tot_ps = psum_pool.tile([1, 1], F32)
        nc.tensor.matmul(tot_ps[:], ones[:], rs[:], start=True, stop=True)
        tot = spool.tile([1, 1], F32)
        nc.vector.tensor_copy(tot[:], tot_ps[:])
        nc.sync.dma_start(out=out[:, :], in_=tot[:])

    nc.finalize()
    return nc


_NC = None


def _get_nc() -> bass.Bass:
    global _NC
    if _NC is None:
        _NC = _build()
    return _NC


def _to_bf16(a: np.ndarray) -> np.ndarray:
    import ml_dtypes

    return a.astype(ml_dtypes.bfloat16)


def make_in_maps(input: np.ndarray, target: np.ndarray) -> list[dict]:
    x = np.ascontiguousarray(np.asarray(input, dtype=np.float32)).reshape(
        N_CORES, RB, 128, C
    )
    if not X_F32:
        x = _to_bf16(x)
    tf = np.asarray(target).astype(np.float64)
    n2 = C * (tf - MU) ** 2 + K
    bias = np.log(COEFF) - 0.5 * np.log(n2)
    bias = np.where(tf == 0, -1e4, bias).astype(np.float32)
    tv = tf.astype(np.float32).reshape(N_CORES, RB, 128)
    bv = bias.reshape(N_CORES, RB, 128)
    cols = [tv, tv - 2048.0, tv - 4096.0, bv]
    aux = np.concatenate([c.transpose(0, 2, 1) for c in cols], axis=2)
    aux = np.ascontiguousarray(aux, dtype=np.float32)
    return [{"x": x[i], "aux": aux[i]} for i in range(N_CORES)]


def run(input: np.ndarray, target: np.ndarray, trace: bool = False, tmpdir=None):
    nc = _get_nc()
    in_maps = make_in_maps(input, target)
    res = run_bass_kernel_spmd(
        nc, in_maps, list(range(N_CORES)), trace=trace, tmpdir=tmpdir
    )
    total = np.float32(0.0)
    for r in res.results:
        total += np.float32(np.sum(np.asarray(r["out"], dtype=np.float32)))
    return np.asarray(total, dtype=np.float32), res


def kernel(input: np.ndarray, target: np.ndarray) -> np.ndarray:
    out, _ = run(input, target)
    return out


# revision 27
# speedup vs baseline: 1.4286x; 1.0077x over previous
"""DistanceFromAnswerLoss on 8 Trainium2 NeuronCores — fused custom-DVE op, v2.

out = 0.1 * sum_{b,c} mask[b,c] * exp(input[b,c])
  mask[b,c] = |c - t_b| / sqrt(sum_c (c - t_b)^2),  mask = 0 where t_b == 0

Per-core pipeline:
  host    : x cast to bf16; bias_b = ln(0.1) - 0.5*ln(C*(t-mu)^2 + K)
            (-1e4 for t==0); aux also carries t-2048 and t-4096 so every
            fused call indexes the same iota[0:4096] window.
  ScalarE : a dummy [128,1] exp FIRST in program order so the activation
            table loads before the x stream saturates the DMA rings;
            then e' = exp(x + bias_b) -> bf16 per tile.
  VectorE : ONE fused op per tile: out = max(iota - t', t' - iota) * e',
            accum_out = rowsum(out)  — |c-t|*e' and the row reduction in
            a single pass.  Tile 0 is split 2x2048 so the chain starts as
            soon as the gpsimd iota seed lands; the only iota expansion
            ([2048:4096], one 4x tensor_scalar) hides between fused calls.
  TensorE : ones-matmul for the final partition reduce -> [1,1] scalar.
"""

import os
import sys
from contextlib import ExitStack

import numpy as np

sys.path.insert(0, "/opt/trn_rl_repo")

import concourse.bass as bass
import concourse.tile as tile
from concourse import bacc, mybir
from concourse.bass_utils import run_bass_kernel_spmd

B = 4096
C = 8192
N_CORES = 8
ROWS = B // N_CORES
RB = ROWS // 128
W = 4096
NW = C // W
NT = RB * NW                 # 8 tile-equivalents per core
IW = 1024                    # gpsimd iota seed width; DVE-expanded to 4096
NACC = 10                    # one accumulator column per fused call
X_F32 = bool(os.environ.get("X_F32"))
COEFF = 0.1

MU = (C - 1) / 2.0
_S1 = (C - 1) * C // 2
_S2 = (C - 1) * C * (2 * C - 1) // 6
K = float(_S2 - _S1 * _S1 / C)

F32 = mybir.dt.float32
BF16 = mybir.dt.bfloat16
Af = mybir.ActivationFunctionType
Op = mybir.AluOpType
XDT = F32 if X_F32 else BF16

_OP_NAME = "ABSDIFF_MUL_REDUCE_ANT"


def _register_dist_op():
    """out = max(Src0 - s0, s0 - Src0) * Src1 ; accum_out = rowsum(out)."""
    from operator import add

    from concourse import dve_ops
    from concourse.dve_spec import C0, Spec, Src0, Src1, lower, maxx
    from concourse.dve_uop import DveOpSpec

    for op in dve_ops.OPS:
        if op.name == _OP_NAME:
            return op

    def _ref(in0, in1, s0, s1, imm2):
        s = np.asarray(s0, dtype=np.float64)
        d = np.abs(in0.astype(np.float64) - s)
        b = (d * in1.astype(np.float64)).astype(np.float32)
        acc = (
            b.reshape(b.shape[0], -1).astype(np.float64).sum(-1, keepdims=True)
        ).astype(np.float32)
        return b, acc

    spec = Spec(
        body=maxx(Src0 - C0, C0 - Src0) * Src1, accum=add,
        reference=_ref,
    )
    row = dve_ops._CUSTOM_DVE_ROW_BASE + len(dve_ops.OPS)
    shas = {
        ver: DveOpSpec(
            name=_OP_NAME, opcode=row, uops=lower(spec, ver=ver), rd1_en=True
        ).sha(ver)
        for ver in ("v3", "v4")
    }
    op = dve_ops.DveOp(_OP_NAME, spec, subdim=False, uops_sha=shas)
    dve_ops.OPS.append(op)
    dve_ops.CUSTOM_DVE_SPECS[op.name] = spec
    dve_ops._SUB_OPCODE_FOR_NAME[op.name] = row
    return op


# schedule: (rb, c0, width, scalar-group); group g holds t - G_OFF[g] so a
# call of width wd always reads iota[0:wd].  Row-block 0 ramps 1k/1k/2k/4k
# so the fused chain starts as soon as the 1k gpsimd iota seed lands.
G_OFF = (0, 1024, 2048, 4096)


def _schedule():
    s = [(0, 0, 1024, 0), (0, 1024, 1024, 1), (0, 2048, 2048, 2),
         (0, 4096, 4096, 3)]
    for rb in range(1, RB):
        s.append((rb, 0, W, 0))
        s.append((rb, W, W, 3))
    return s


def _build() -> bass.Bass:
    dist_op = _register_dist_op()
    nc = bacc.Bacc("TRN2", target_bir_lowering=False, debug=False)
    x = nc.declare_dram_parameter("x", [RB, 128, C], XDT, isOutput=False)
    # aux columns: [t | t-1024 | t-2048 | t-4096 | bias], 4 rbs each
    aux = nc.declare_dram_parameter("aux", [128, 5 * RB], F32, isOutput=False)
    out = nc.declare_dram_parameter("out", [1, 1], F32, isOutput=True)

    sched = _schedule()

    with tile.TileContext(nc) as tc, ExitStack() as ctx:
        const_pool = ctx.enter_context(tc.tile_pool(name="const", bufs=1))
        xpool = ctx.enter_context(tc.tile_pool(name="x", bufs=5))
        epool = ctx.enter_context(tc.tile_pool(name="e", bufs=4))
        dpool = ctx.enter_context(tc.tile_pool(name="d", bufs=2))
        spool = ctx.enter_context(tc.tile_pool(name="s", bufs=1))
        psum_pool = ctx.enter_context(tc.tile_pool(name="ps", bufs=1, space="PSUM"))

        # --- activation-table preload: dummy exp BEFORE any DMA -----------
        dum0 = const_pool.tile([128, 1], F32)
        nc.vector.memset(dum0[:], 0.0)
        dum1 = const_pool.tile([128, 1], BF16)
        nc.scalar.activation(dum1[:], dum0[:], Af.Exp)

        # --- sync queue: aux first (it gates bcol0 -> exp0), then x -------
        auxt = const_pool.tile([128, 5 * RB], F32)
        nc.sync.dma_start(out=auxt[:], in_=aux[:, :])
        xts = {}
        for si in range(3):
            rb, c0, wd, _ = sched[si]
            xt = xpool.tile([128, wd], XDT, tag=f"x{wd}")
            nc.sync.dma_start(out=xt[:], in_=x[rb, :, c0:c0 + wd])
            xts[si] = xt

        # ALL per-row scalar tiles are copied on the (otherwise idle)
        # ScalarE queue: the ones the head of the pipeline needs right
        # away here, the rest interleaved between exps in the main loop
        # (each pair costs ~0.6us of Sc slack, never pacing the chain).
        bcols, tg = [], {}
        for rb in range(RB):
            bc = const_pool.tile([128, 1], F32, tag=f"bc{rb}")
            bcols.append(bc)
        used = sorted({(g, rb) for rb, _, _, g in sched})
        for g, rb in used:
            t_ = const_pool.tile([128, 1], F32, tag=f"t{g}_{rb}")
            tg[(g, rb)] = t_

        def sc_copy_bcol(rb):
            nc.scalar.copy(bcols[rb][:], auxt[:, 4 * RB + rb:4 * RB + rb + 1])

        def sc_copy_tg(g, rb):
            nc.scalar.copy(tg[(g, rb)][:], auxt[:, g * RB + rb:g * RB + rb + 1])

        sc_copy_bcol(0)
        sc_copy_tg(0, 0)
        sc_copy_tg(1, 0)
        # remaining copies, interleaved after exp_si in the main loop; each
        # lands well before its first consumer (checked against the chain)
        deferred = {
            1: [("t", 2, 0)],
            2: [("t", 3, 0)],
            3: [("b", 1), ("t", 0, 1)],
            4: [("b", 2), ("t", 3, 1)],
            5: [("t", 0, 2)],
            6: [("b", 3), ("t", 3, 2)],
            7: [("t", 0, 3)],
            8: [("t", 3, 3)],
        }
        ones = const_pool.tile([128, 1], F32)
        nc.vector.memset(ones[:], 1.0)

        acc = spool.tile([128, NACC], F32)

        iota = const_pool.tile([128, W], BF16)
        nc.gpsimd.iota(
            iota[:, 0:IW], pattern=[[1, IW]], base=0, channel_multiplier=0,
            allow_small_or_imprecise_dtypes=True,
        )

        def fused(si, et, rb, width, g):
            dm = dpool.tile([128, width], BF16, tag=f"dm{width}")
            nc.vector._custom_dve(
                dist_op, out=dm[:], in0=iota[:, 0:width], in1=et[:],
                s0=tg[(g, rb)][:], accum_out=acc[:, si:si + 1],
            )

        ets = {}
        for si, (rb, c0, wd, g) in enumerate(sched):
            if si not in xts:
                xt = xpool.tile([128, wd], XDT, tag=f"x{wd}")
                nc.sync.dma_start(out=xt[:], in_=x[rb, :, c0:c0 + wd])
            else:
                xt = xts[si]
            et = epool.tile([128, wd], BF16, tag=f"e{wd}")
            nc.scalar.activation(et[:], xt[:], Af.Exp, bias=bcols[rb][:])
            ets[si] = et
            fused(si, et, rb, wd, g)
            for item in deferred.get(si, ()):
                if item[0] == "b":
                    sc_copy_bcol(item[1])
                else:
                    sc_copy_tg(item[1], item[2])
            # iota doublings hidden between fused calls: [1024:2048] after
            # f1 (first needed by f2), [2048:4096] after f2 (needed by f3)
            if si == 1:
                nc.vector.tensor_scalar(
                    iota[:, IW:2 * IW], iota[:, 0:IW], float(IW), None,
                    op0=Op.add,
                )
            elif si == 2:
                nc.vector.tensor_scalar(
                    iota[:, 2 * IW:4 * IW], iota[:, 0:2 * IW], float(2 * IW),
                    None, op0=Op.add,
                )

        # --- combine: rs = rowsum(acc); tot = ones . rs -> [1,1] ----------
        rs = spool.tile([128, 1], F32)
        nc.vector.tensor_reduce(
            rs[:], acc[:], axis=mybir.AxisListType.X, op=Op.add
        )
        tot_ps = psum_pool.tile([1, 1], F32)
        nc.tensor.matmul(tot_ps[:], ones[:], rs[:], start=True, stop=True)
        tot = spool.tile([1, 1], F32)
        nc.vector.tensor_copy(tot[:], tot_ps[:])
        nc.sync.dma_start(out=out[:, :], in_=tot[:])

    nc.finalize()
    return nc


_NC = None


def _get_nc() -> bass.Bass:
    global _NC
    if _NC is None:
        _NC = _build()
    return _NC


def _to_bf16(a: np.ndarray) -> np.ndarray:
    import ml_dtypes

    return a.astype(ml_dtypes.bfloat16)


def make_in_maps(input: np.ndarray, target: np.ndarray) -> list[dict]:
    x = np.ascontiguousarray(np.asarray(input, dtype=np.float32)).reshape(
        N_CORES, RB, 128, C
    )
    if not X_F32:
        x = _to_bf16(x)
    tf = np.asarray(target).astype(np.float64)
    n2 = C * (tf - MU) ** 2 + K
    bias = np.log(COEFF) - 0.5 * np.log(n2)
    bias = np.where(tf == 0, -1e4, bias).astype(np.float32)
    tv = tf.astype(np.float32).reshape(N_CORES, RB, 128)
    bv = bias.reshape(N_CORES, RB, 128)
    cols = [tv - o for o in G_OFF] + [bv]
    aux = np.concatenate([c.transpose(0, 2, 1) for c in cols], axis=2)
    aux = np.ascontiguousarray(aux, dtype=np.float32)
    return [{"x": x[i], "aux": aux[i]} for i in range(N_CORES)]


def run(input: np.ndarray, target: np.ndarray, trace: bool = False, tmpdir=None):
    nc = _get_nc()
    in_maps = make_in_maps(input, target)
    res = run_bass_kernel_spmd(
        nc, in_maps, list(range(N_CORES)), trace=trace, tmpdir=tmpdir
    )
    total = np.float32(0.0)
    for r in res.results:
        total += np.float32(np.sum(np.asarray(r["out"], dtype=np.float32)))
    return np.asarray(total, dtype=np.float32), res


def kernel(input: np.ndarray, target: np.ndarray) -> np.ndarray:
    out, _ = run(input, target)
    return out


# revision 28
# speedup vs baseline: 1.4297x; 1.0007x over previous
"""DistanceFromAnswerLoss on 8 Trainium2 NeuronCores — fused custom-DVE op, v2.

out = 0.1 * sum_{b,c} mask[b,c] * exp(input[b,c])
  mask[b,c] = |c - t_b| / sqrt(sum_c (c - t_b)^2),  mask = 0 where t_b == 0

Per-core pipeline:
  host    : x cast to bf16; bias_b = ln(0.1) - 0.5*ln(C*(t-mu)^2 + K)
            (-1e4 for t==0); aux also carries t-2048 and t-4096 so every
            fused call indexes the same iota[0:4096] window.
  ScalarE : a dummy [128,1] exp FIRST in program order so the activation
            table loads before the x stream saturates the DMA rings;
            then e' = exp(x + bias_b) -> bf16 per tile.
  VectorE : ONE fused op per tile: out = max(iota - t', t' - iota) * e',
            accum_out = rowsum(out)  — |c-t|*e' and the row reduction in
            a single pass.  Tile 0 is split 2x2048 so the chain starts as
            soon as the gpsimd iota seed lands; the only iota expansion
            ([2048:4096], one 4x tensor_scalar) hides between fused calls.
  TensorE : ones-matmul for the final partition reduce -> [1,1] scalar.
"""

import os
import sys
from contextlib import ExitStack

import numpy as np

sys.path.insert(0, "/opt/trn_rl_repo")

import concourse.bass as bass
import concourse.tile as tile
from concourse import bacc, mybir
from concourse.bass_utils import run_bass_kernel_spmd

B = 4096
C = 8192
N_CORES = 8
ROWS = B // N_CORES
RB = ROWS // 128
W = 4096
NW = C // W
NT = RB * NW                 # 8 tile-equivalents per core
IW = 1024                    # gpsimd iota seed width; DVE-expanded to 4096
NACC = 10                    # one accumulator column per fused call
X_F32 = bool(os.environ.get("X_F32"))
COEFF = 0.1

MU = (C - 1) / 2.0
_S1 = (C - 1) * C // 2
_S2 = (C - 1) * C * (2 * C - 1) // 6
K = float(_S2 - _S1 * _S1 / C)

F32 = mybir.dt.float32
BF16 = mybir.dt.bfloat16
Af = mybir.ActivationFunctionType
Op = mybir.AluOpType
XDT = F32 if X_F32 else BF16

_OP_NAME = "ABSDIFF_MUL_REDUCE_ANT"


def _register_dist_op():
    """out = max(Src0 - s0, s0 - Src0) * Src1 ; accum_out = rowsum(out)."""
    from operator import add

    from concourse import dve_ops
    from concourse.dve_spec import C0, Spec, Src0, Src1, lower, maxx
    from concourse.dve_uop import DveOpSpec

    for op in dve_ops.OPS:
        if op.name == _OP_NAME:
            return op

    def _ref(in0, in1, s0, s1, imm2):
        s = np.asarray(s0, dtype=np.float64)
        d = np.abs(in0.astype(np.float64) - s)
        b = (d * in1.astype(np.float64)).astype(np.float32)
        acc = (
            b.reshape(b.shape[0], -1).astype(np.float64).sum(-1, keepdims=True)
        ).astype(np.float32)
        return b, acc

    spec = Spec(
        body=maxx(Src0 - C0, C0 - Src0) * Src1, accum=add,
        reference=_ref,
    )
    row = dve_ops._CUSTOM_DVE_ROW_BASE + len(dve_ops.OPS)
    shas = {
        ver: DveOpSpec(
            name=_OP_NAME, opcode=row, uops=lower(spec, ver=ver), rd1_en=True
        ).sha(ver)
        for ver in ("v3", "v4")
    }
    op = dve_ops.DveOp(_OP_NAME, spec, subdim=False, uops_sha=shas)
    dve_ops.OPS.append(op)
    dve_ops.CUSTOM_DVE_SPECS[op.name] = spec
    dve_ops._SUB_OPCODE_FOR_NAME[op.name] = row
    return op


# schedule: (rb, c0, width, scalar-group); group g holds t - G_OFF[g] so a
# call of width wd always reads iota[0:wd].  Row-block 0 ramps 1k/1k/2k/4k
# so the fused chain starts as soon as the 1k gpsimd iota seed lands.
G_OFF = (0, 1024, 2048, 4096)


def _schedule():
    s = [(0, 0, 1024, 0), (0, 1024, 1024, 1), (0, 2048, 2048, 2),
         (0, 4096, 4096, 3)]
    for rb in range(1, RB):
        s.append((rb, 0, W, 0))
        s.append((rb, W, W, 3))
    return s


def _build() -> bass.Bass:
    dist_op = _register_dist_op()
    nc = bacc.Bacc("TRN2", target_bir_lowering=False, debug=False)
    x = nc.declare_dram_parameter("x", [RB, 128, C], XDT, isOutput=False)
    # aux columns: [t | t-1024 | t-2048 | t-4096 | bias], 4 rbs each
    aux = nc.declare_dram_parameter("aux", [128, 5 * RB], F32, isOutput=False)
    out = nc.declare_dram_parameter("out", [1, 1], F32, isOutput=True)

    sched = _schedule()

    with tile.TileContext(nc) as tc, ExitStack() as ctx:
        const_pool = ctx.enter_context(tc.tile_pool(name="const", bufs=1))
        xpool = ctx.enter_context(tc.tile_pool(name="x", bufs=5))
        epool = ctx.enter_context(tc.tile_pool(name="e", bufs=4))
        dpool = ctx.enter_context(tc.tile_pool(name="d", bufs=2))
        spool = ctx.enter_context(tc.tile_pool(name="s", bufs=1))
        psum_pool = ctx.enter_context(tc.tile_pool(name="ps", bufs=1, space="PSUM"))

        # --- activation-table preload: dummy exp BEFORE any DMA -----------
        dum0 = const_pool.tile([128, 1], F32)
        nc.vector.memset(dum0[:], 0.0)
        dum1 = const_pool.tile([128, 1], BF16)
        nc.scalar.activation(dum1[:], dum0[:], Af.Exp)

        # --- sync queue: aux first (it gates bcol0 -> exp0), then x -------
        auxt = const_pool.tile([128, 5 * RB], F32)
        nc.sync.dma_start(out=auxt[:], in_=aux[:, :])
        xts = {}
        for si in range(3):
            rb, c0, wd, _ = sched[si]
            xt = xpool.tile([128, wd], XDT, tag=f"x{wd}")
            nc.sync.dma_start(out=xt[:], in_=x[rb, :, c0:c0 + wd])
            xts[si] = xt

        # ALL per-row scalar tiles are copied on the (otherwise idle)
        # ScalarE queue: the ones the head of the pipeline needs right
        # away here, the rest interleaved between exps in the main loop
        # (each pair costs ~0.6us of Sc slack, never pacing the chain).
        bcols, tg = [], {}
        for rb in range(RB):
            bc = const_pool.tile([128, 1], F32, tag=f"bc{rb}")
            bcols.append(bc)
        used = sorted({(g, rb) for rb, _, _, g in sched})
        for g, rb in used:
            t_ = const_pool.tile([128, 1], F32, tag=f"t{g}_{rb}")
            tg[(g, rb)] = t_

        def sc_copy_bcol(rb):
            nc.scalar.copy(bcols[rb][:], auxt[:, 4 * RB + rb:4 * RB + rb + 1])

        def sc_copy_tg(g, rb):
            nc.scalar.copy(tg[(g, rb)][:], auxt[:, g * RB + rb:g * RB + rb + 1])

        # ScalarE carries only the 4 bias copies (the static scheduler
        # front-loads whatever sits on the Sc queue ahead of exp0, so keep
        # that set minimal); all t copies ride the DVE pre-chain idle window.
        sc_copy_bcol(0)
        deferred = {0: [("b", 1), ("b", 2), ("b", 3)]}
        for g, rb in used:
            nc.vector.tensor_copy(
                tg[(g, rb)][:], auxt[:, g * RB + rb:g * RB + rb + 1]
            )
        ones = const_pool.tile([128, 1], F32)
        nc.vector.memset(ones[:], 1.0)

        acc = spool.tile([128, NACC], F32)

        iota = const_pool.tile([128, W], BF16)
        nc.gpsimd.iota(
            iota[:, 0:IW], pattern=[[1, IW]], base=0, channel_multiplier=0,
            allow_small_or_imprecise_dtypes=True,
        )

        def fused(si, et, rb, width, g):
            dm = dpool.tile([128, width], BF16, tag=f"dm{width}")
            nc.vector._custom_dve(
                dist_op, out=dm[:], in0=iota[:, 0:width], in1=et[:],
                s0=tg[(g, rb)][:], accum_out=acc[:, si:si + 1],
            )

        ets = {}
        for si, (rb, c0, wd, g) in enumerate(sched):
            if si not in xts:
                xt = xpool.tile([128, wd], XDT, tag=f"x{wd}")
                nc.sync.dma_start(out=xt[:], in_=x[rb, :, c0:c0 + wd])
            else:
                xt = xts[si]
            et = epool.tile([128, wd], BF16, tag=f"e{wd}")
            nc.scalar.activation(et[:], xt[:], Af.Exp, bias=bcols[rb][:])
            ets[si] = et
            fused(si, et, rb, wd, g)
            for item in deferred.get(si, ()):
                if item[0] == "b":
                    sc_copy_bcol(item[1])
                else:
                    sc_copy_tg(item[1], item[2])
            # iota doublings hidden between fused calls: [1024:2048] after
            # f1 (first needed by f2), [2048:4096] after f2 (needed by f3)
            if si == 1:
                nc.vector.tensor_scalar(
                    iota[:, IW:2 * IW], iota[:, 0:IW], float(IW), None,
                    op0=Op.add,
                )
            elif si == 2:
                nc.vector.tensor_scalar(
                    iota[:, 2 * IW:4 * IW], iota[:, 0:2 * IW], float(2 * IW),
                    None, op0=Op.add,
                )

        # --- combine: rs = rowsum(acc); tot = ones . rs -> [1,1] ----------
        rs = spool.tile([128, 1], F32)
        nc.vector.tensor_reduce(
            rs[:], acc[:], axis=mybir.AxisListType.X, op=Op.add
        )
        tot_ps = psum_pool.tile([1, 1], F32)
        nc.tensor.matmul(tot_ps[:], ones[:], rs[:], start=True, stop=True)
        tot = spool.tile([1, 1], F32)
        nc.vector.tensor_copy(tot[:], tot_ps[:])
        nc.sync.dma_start(out=out[:, :], in_=tot[:])

    nc.finalize()
    return nc


_NC = None


def _get_nc() -> bass.Bass:
    global _NC
    if _NC is None:
        _NC = _build()
    return _NC


def _to_bf16(a: np.ndarray) -> np.ndarray:
    import ml_dtypes

    return a.astype(ml_dtypes.bfloat16)


def make_in_maps(input: np.ndarray, target: np.ndarray) -> list[dict]:
    x = np.ascontiguousarray(np.asarray(input, dtype=np.float32)).reshape(
        N_CORES, RB, 128, C
    )
    if not X_F32:
        x = _to_bf16(x)
    tf = np.asarray(target).astype(np.float64)
    n2 = C * (tf - MU) ** 2 + K
    bias = np.log(COEFF) - 0.5 * np.log(n2)
    bias = np.where(tf == 0, -1e4, bias).astype(np.float32)
    tv = tf.astype(np.float32).reshape(N_CORES, RB, 128)
    bv = bias.reshape(N_CORES, RB, 128)
    cols = [tv - o for o in G_OFF] + [bv]
    aux = np.concatenate([c.transpose(0, 2, 1) for c in cols], axis=2)
    aux = np.ascontiguousarray(aux, dtype=np.float32)
    return [{"x": x[i], "aux": aux[i]} for i in range(N_CORES)]


def run(input: np.ndarray, target: np.ndarray, trace: bool = False, tmpdir=None):
    nc = _get_nc()
    in_maps = make_in_maps(input, target)
    res = run_bass_kernel_spmd(
        nc, in_maps, list(range(N_CORES)), trace=trace, tmpdir=tmpdir
    )
    total = np.float32(0.0)
    for r in res.results:
        total += np.float32(np.sum(np.asarray(r["out"], dtype=np.float32)))
    return np.asarray(total, dtype=np.float32), res


def kernel(input: np.ndarray, target: np.ndarray) -> np.ndarray:
    out, _ = run(input, target)
    return out
